# revision 2
# baseline (speedup 1.0000x reference)
"""CrystalTransformer (TransformerConv x3 + segment-mean pool) on 8 trn2 cores.

v2: hardware-loop restructure of the one-hot-scatter kernel. Per core:
20 dst blocks x tpb 128-edge tiles. For_i loops over blocks for embed/B1/B2;
only the per-tile indirect h[src] gathers are python-unrolled (indirect DMA
doesn't compile inside For_i on this walrus). q is broadcast edge-wise on-chip
via qt = (S^T)^T @ q_blk (S = one-hot dst_rel matrix, transposed on PE), so no
q gather and no q DRAM round-trip. V is projected per edge (ve = X1 @ Wv2) and
the scatter payload is [ve*ex per head | ex] (260 cols), which removes the
per-head transpose+matmul combine of the old B3. Edge features ship only as
eaT (feature-major, bf16); weights bf16; h stays f32.
"""
import json
import numpy as np
import ml_dtypes

P = 128
N, E, G = 20000, 320000, 256
DA, DE, D, H, L = 92, 50, 64, 4, 3
NCORES = 8
NLOC = 2560            # node slots per core (20 blocks of 128)
NB = NLOC // P         # 20 dst blocks per core
NPAD = NLOC * NCORES   # 20480
XW = D + DE + 1        # 115 = [h_src(64) | ea(50) | 1]
ZW = H * D + H         # 260 = [ve*ex per head | ex]


# ---------------------------------------------------------------- BIR patch --
def _install_birpatch():
    """This container's walrus rejects >1 sem wait per instruction; hoist
    extras onto injected preceding Drains (same engine => same order)."""
    import concourse.bass2jax as b2j
    if getattr(b2j, "_birpatch_installed", False):
        return
    orig = b2j.compile_bir_kernel

    def patch(bir_bytes):
        d = json.loads(bir_bytes)
        for fn in d.get("functions", []):
            for blk in fn.get("blocks", []):
                out = []
                for ins in blk.get("instructions", []):
                    si = ins.get("sync_info") or {}
                    waits = si.get("on_wait") or []
                    if len(waits) > 1:
                        for k, w in enumerate(waits[:-1]):
                            out.append({
                                "debug": ins.get("debug", 0),
                                "engine": ins["engine"], "ins": [], "outs": [],
                                "name": f'{ins["name"]}-w{k}', "opcode": "Drain",
                                "sync_info": {"on_update": [], "on_wait": [w]},
                            })
                        si["on_wait"] = waits[-1:]
                    out.append(ins)
                blk["instructions"] = out
        return json.dumps(d).encode()

    def wrapper(bir_str, *a, **kw):
        try:
            bir_str = patch(bir_str)
        except Exception as e:  # pragma: no cover
            print("[birpatch] failed:", e)
        return orig(bir_str, *a, **kw)

    b2j.compile_bir_kernel = wrapper
    b2j._birpatch_installed = True


# ------------------------------------------------------------------- device --
def _build_nc(tpb):
    import concourse.bass as bass
    import concourse.mybir as mybir
    import concourse.tile as tile
    from concourse.bass import ds
    from concourse.masks import make_identity

    f32, i32, bf16 = mybir.dt.float32, mybir.dt.int32, mybir.dt.bfloat16
    Alu, Act = mybir.AluOpType, mybir.ActivationFunctionType
    NT = NB * tpb          # edge tiles per core
    EB = NT * P            # padded edges per core
    TB = tpb * P           # edges per block

    nc = bass.Bass("TRN2", target_bir_lowering=False, debug=False,
                   num_devices=NCORES)
    di = lambda nm, sh, dt=f32: nc.dram_tensor(nm, sh, dt, kind="ExternalInput")
    x_in = di("x_shard", [NLOC, DA])
    eaT_in = di("eaT_pad", [DE + 1, EB], bf16)
    srcidx_in = di("srcidx", [P, NT], i32)
    metmask_in = di("metmask", [P, 2 * NT])       # [dst_rel | mask]
    brel_in = di("batch_rel", [NLOC, 1])
    watom_in = di("w_atom_aug", [DA + 1, D])
    w2k_in = di("w2k", [L, XW, H * D], bf16)
    wv2_in = di("wv2", [L, XW, H * D], bf16)
    wqs_in = di("wqs", [L, D + 1, H * D + D])
    out_pool = nc.dram_tensor("out_pool", [P, D + 1], f32, kind="ExternalOutput")

    h_mine = nc.dram_tensor("h_mine", [NLOC, D], f32)
    h_full = [nc.dram_tensor(f"h_full_{l}", [NPAD, D], f32, addr_space="Shared")
              for l in range(L)]

    with tile.TileContext(nc, num_cores=NCORES) as tc:
        import contextlib
        with contextlib.ExitStack() as st:
            cp = st.enter_context(tc.tile_pool(name="const", bufs=1))
            io = st.enter_context(tc.tile_pool(name="io", bufs=3))
            xp = st.enter_context(tc.tile_pool(name="xt", bufs=3))
            vp = st.enter_context(tc.tile_pool(name="dve", bufs=3))
            ps_tr = st.enter_context(tc.tile_pool(name="ps_tr", bufs=2, space="PSUM"))
            ps_ke = st.enter_context(tc.tile_pool(name="ps_ke", bufs=1, space="PSUM"))
            ps_ve = st.enter_context(tc.tile_pool(name="ps_ve", bufs=1, space="PSUM"))
            ps_qt = st.enter_context(tc.tile_pool(name="ps_qt", bufs=1, space="PSUM"))
            ps_z = st.enter_context(tc.tile_pool(name="ps_z", bufs=2, space="PSUM"))
            ps_b = st.enter_context(tc.tile_pool(name="ps_b", bufs=1, space="PSUM"))

            ident = cp.tile([P, P], f32)
            make_identity(nc, ident[:])
            iota_i = cp.tile([P, P], i32)
            nc.gpsimd.iota(iota_i[:], pattern=[[1, P]], base=0, channel_multiplier=0)
            iota_f = cp.tile([P, P], f32)
            nc.vector.tensor_copy(iota_f[:], iota_i[:])
            ones_col = cp.tile([P, 1], f32)
            nc.vector.memset(ones_col[:], 1.0)
            srcidx = cp.tile([P, NT], i32)
            nc.sync.dma_start(out=srcidx[:], in_=srcidx_in[:])
            metmask = cp.tile([P, 2 * NT], f32)
            nc.sync.dma_start(out=metmask[:], in_=metmask_in[:])
            watom_sb = cp.tile([DA + 1, D], f32)
            nc.sync.dma_start(out=watom_sb[:], in_=watom_in[:])
            X_all = cp.tile([P, NT * D], f32)         # 87KB/part

            # ---- embed: h0 = x@W_atom + b
            with tc.For_i(0, NB, 1) as b:
                xb = io.tile([P, DA], f32, tag="xb")
                nc.sync.dma_start(out=xb[:], in_=x_in[ds(b * P, P), :])
                xT_ps = ps_tr.tile([DA, P], f32, tag="tr")
                nc.tensor.transpose(out=xT_ps[:], in_=xb[:], identity=ident[:])
                xT = xp.tile([DA + 1, P], f32, tag="xat")
                nc.vector.memset(xT[:], 1.0)
                nc.vector.tensor_copy(xT[:DA, :], xT_ps[:])
                hb_ps = ps_b.tile([P, H * D + D], f32, tag="blk")
                nc.tensor.matmul(hb_ps[:, :D], lhsT=xT[:], rhs=watom_sb[:],
                                 start=True, stop=True)
                hb = vp.tile([P, D], f32, tag="hb")
                nc.vector.tensor_copy(hb[:], hb_ps[:, :D])
                nc.sync.dma_start(out=h_mine[ds(b * P, P), :], in_=hb[:])
            tc.strict_bb_all_engine_barrier()
            nc.gpsimd.collective_compute(
                "AllGather", Alu.bypass,
                replica_groups=[list(range(NCORES))],
                ins=[h_mine.ap().opt()], outs=[h_full[0].ap().opt()])
            tc.strict_bb_all_engine_barrier()

            for l in range(L):
                w2k_sb = cp.tile([XW, H * D], bf16, tag="w2k")
                nc.sync.dma_start(out=w2k_sb[:], in_=w2k_in[l])
                wv2_sb = cp.tile([XW, H * D], bf16, tag="wv2")
                nc.sync.dma_start(out=wv2_sb[:], in_=wv2_in[l])
                wqs_sb = cp.tile([D + 1, H * D + D], f32, tag="wqs")
                nc.sync.dma_start(out=wqs_sb[:], in_=wqs_in[l])

                # ---- G: gather h[src] per edge tile (HW honors 128 offsets
                # per indirect DMA; multi-column offset APs silently gather
                # only the first column's rows)
                for t in range(NT):
                    nc.gpsimd.indirect_dma_start(
                        out=X_all[:, t * D:(t + 1) * D], out_offset=None,
                        in_=h_full[l][:],
                        in_offset=bass.IndirectOffsetOnAxis(
                            ap=srcidx[:, t:t + 1], axis=0))

                # ---- B2: q/skip + edge tiles + combine, per block
                with tc.For_i(0, NB, 1) as b:
                    hblk = io.tile([P, D], f32, tag="hblk")
                    nc.sync.dma_start(out=hblk[:], in_=h_mine[ds(b * P, P), :])
                    hT_ps = ps_tr.tile([D, P], f32, tag="tr")
                    nc.tensor.transpose(out=hT_ps[:], in_=hblk[:], identity=ident[:])
                    hT = xp.tile([D + 1, P], f32, tag="hat")
                    nc.vector.memset(hT[:], 1.0)
                    nc.vector.tensor_copy(hT[:D, :], hT_ps[:])
                    qs_ps = ps_b.tile([P, H * D + D], f32, tag="blk")
                    nc.tensor.matmul(qs_ps[:], lhsT=hT[:], rhs=wqs_sb[:],
                                     start=True, stop=True)
                    qb = xp.tile([P, H * D], f32, tag="qb")
                    nc.scalar.copy(qb[:], qs_ps[:, :H * D])
                    skip_b = xp.tile([P, D], f32, tag="skip_b")
                    nc.scalar.copy(skip_b[:], qs_ps[:, H * D:])
                    mm_blk = vp.tile([P, 2 * tpb], f32, tag="mm_blk")
                    nc.vector.tensor_copy(
                        mm_blk[:], metmask[:, ds(b * (2 * tpb), 2 * tpb)])
                    X_blk = vp.tile([P, tpb * D], f32, tag="X_blk")
                    nc.vector.tensor_copy(
                        X_blk[:], X_all[:, ds(b * (tpb * D), tpb * D)])
                    eaT_blk = io.tile([DE + 1, TB], bf16, tag="eaT_blk")
                    nc.sync.dma_start(out=eaT_blk[:],
                                      in_=eaT_in[:, ds(b * TB, TB)])
                    z_ps = ps_z.tile([P, ZW], f32, tag="z")
                    for t in range(tpb):
                        XT = xp.tile([XW, P], bf16, tag="XT")
                        nc.scalar.copy(XT[D:, :], eaT_blk[:, t * P:(t + 1) * P])
                        hsT_ps = ps_tr.tile([D, P], f32, tag="tr")
                        nc.tensor.transpose(out=hsT_ps[:],
                                            in_=X_blk[:, t * D:(t + 1) * D],
                                            identity=ident[:])
                        nc.vector.tensor_copy(XT[:D, :], hsT_ps[:])
                        ke_ps = ps_ke.tile([P, H * D], f32, tag="ke")
                        nc.tensor.matmul(ke_ps[:], lhsT=XT[:], rhs=w2k_sb[:],
                                         start=True, stop=True)
                        ve_ps = ps_ve.tile([P, H * D], f32, tag="ve")
                        nc.tensor.matmul(ve_ps[:], lhsT=XT[:], rhs=wv2_sb[:],
                                         start=True, stop=True)
                        S = vp.tile([P, P], f32, tag="S")
                        nc.gpsimd.tensor_scalar(
                            out=S[:], in0=iota_f[:],
                            scalar1=mm_blk[:, 2 * t:2 * t + 1],
                            scalar2=mm_blk[:, 2 * t + 1:2 * t + 2],
                            op0=Alu.is_equal, op1=Alu.mult)
                        ST_ps = ps_tr.tile([P, P], f32, tag="tr")
                        nc.tensor.transpose(out=ST_ps[:], in_=S[:],
                                            identity=ident[:])
                        ST = vp.tile([P, P], f32, tag="ST")
                        nc.scalar.copy(ST[:], ST_ps[:])
                        qt_ps = ps_qt.tile([P, H * D], f32, tag="qt")
                        nc.tensor.matmul(qt_ps[:], lhsT=ST[:], rhs=qb[:],
                                         start=True, stop=True)
                        qt = vp.tile([P, H * D], f32, tag="qtc")
                        nc.scalar.copy(qt[:], qt_ps[:])
                        prod = vp.tile([P, H * D], f32, tag="prod")
                        nc.vector.tensor_tensor(out=prod[:], in0=ke_ps[:],
                                                in1=qt[:], op=Alu.mult)
                        alpha = vp.tile([P, H], f32, tag="alpha")
                        nc.vector.tensor_reduce(
                            out=alpha[:],
                            in_=prod[:].rearrange("p (h d) -> p h d", d=D),
                            axis=mybir.AxisListType.X, op=Alu.add)
                        ex = vp.tile([P, H], f32, tag="ex")
                        nc.scalar.activation(ex[:], alpha[:], Act.Exp,
                                             scale=float(1.0 / np.sqrt(D)))
                        veex = vp.tile([P, ZW], f32, tag="veex")
                        for h in range(H):
                            nc.vector.tensor_scalar_mul(
                                out=veex[:, h * D:(h + 1) * D],
                                in0=ve_ps[:, h * D:(h + 1) * D],
                                scalar1=ex[:, h:h + 1])
                        nc.vector.tensor_copy(veex[:, H * D:], ex[:])
                        nc.tensor.matmul(z_ps[:], lhsT=S[:], rhs=veex[:],
                                         start=(t == 0), stop=(t == tpb - 1))
                    # ---- B3: combine per block
                    den = vp.tile([P, H], f32, tag="den")
                    nc.vector.tensor_scalar_max(out=den[:], in0=z_ps[:, H * D:],
                                                scalar1=1e-30)
                    rden = vp.tile([P, H], f32, tag="rden")
                    nc.vector.reciprocal(rden[:], den[:])
                    zn = vp.tile([P, H * D], f32, tag="zn")
                    for h in range(H):
                        nc.vector.tensor_scalar_mul(
                            out=zn[:, h * D:(h + 1) * D],
                            in0=z_ps[:, h * D:(h + 1) * D],
                            scalar1=rden[:, h:h + 1])
                    agg = vp.tile([P, D], f32, tag="agg")
                    nc.vector.tensor_reduce(
                        out=agg[:],
                        in_=zn[:].rearrange("p (h d) -> p d h", h=H),
                        axis=mybir.AxisListType.X, op=Alu.add)
                    tmp = vp.tile([P, D], f32, tag="tmp")
                    nc.vector.tensor_tensor(out=tmp[:], in0=agg[:],
                                            in1=skip_b[:], op=Alu.add)
                    hb2 = vp.tile([P, D], f32, tag="hb2")
                    nc.vector.tensor_scalar_max(out=hb2[:], in0=tmp[:],
                                                scalar1=0.0)
                    nc.sync.dma_start(out=h_mine[ds(b * P, P), :], in_=hb2[:])
                if l < L - 1:
                    tc.strict_bb_all_engine_barrier()
                    nc.gpsimd.collective_compute(
                        "AllGather", Alu.bypass,
                        replica_groups=[list(range(NCORES))],
                        ins=[h_mine.ap().opt()], outs=[h_full[l + 1].ap().opt()])
                    tc.strict_bb_all_engine_barrier()

            tc.strict_bb_all_engine_barrier()
            # ---- pooling: one-hot on batch ids (python-unrolled, PSUM accum)
            brel = cp.tile([P, NB], f32)
            nc.sync.dma_start(out=brel[:],
                              in_=brel_in[:].rearrange("(b p) o -> p (b o)", p=P))
            pool_ps = ps_z.tile([P, D], f32, tag="z")
            cnt_ps = ps_qt.tile([P, 1], f32, tag="qt")
            for b in range(NB):
                hpb = io.tile([P, D], f32, tag="hpb")
                nc.sync.dma_start(out=hpb[:], in_=h_mine[b * P:(b + 1) * P, :])
                Sb = vp.tile([P, P], f32, tag="S")
                nc.vector.tensor_scalar(out=Sb[:], in0=iota_f[:],
                                        scalar1=brel[:, b:b + 1], scalar2=None,
                                        op0=Alu.is_equal)
                nc.tensor.matmul(pool_ps[:], lhsT=Sb[:], rhs=hpb[:],
                                 start=(b == 0), stop=(b == NB - 1))
                nc.tensor.matmul(cnt_ps[:], lhsT=Sb[:], rhs=ones_col[:],
                                 start=(b == 0), stop=(b == NB - 1),
                                 skip_group_check=True)
            pool_sb = vp.tile([P, D + 1], f32, tag="pool_sb")
            nc.vector.tensor_copy(pool_sb[:, :D], pool_ps[:])
            nc.vector.tensor_copy(pool_sb[:, D:], cnt_ps[:])
            nc.sync.dma_start(out=out_pool[:], in_=pool_sb[:])
    return nc


# --------------------------------------------------------------------- host --
def kernel(**inputs):
    _install_birpatch()
    from concourse.bass_utils import run_bass_kernel_spmd

    x = np.asarray(inputs["x"], np.float32)
    ei = np.asarray(inputs["edge_index"]).astype(np.int64)
    ea = np.asarray(inputs["edge_attr"], np.float32)
    batch = np.asarray(inputs["batch"]).astype(np.int64)
    Wq = np.asarray(inputs["Wq"], np.float32); bq = np.asarray(inputs["bq"], np.float32)
    Wk = np.asarray(inputs["Wk"], np.float32); bk = np.asarray(inputs["bk"], np.float32)
    Wv = np.asarray(inputs["Wv"], np.float32); bv = np.asarray(inputs["bv"], np.float32)
    We = np.asarray(inputs["We"], np.float32)
    Wskip = np.asarray(inputs["Wskip"], np.float32)
    bskip = np.asarray(inputs["bskip"], np.float32)
    W_atom = np.asarray(inputs["W_atom"], np.float32)
    b_atom = np.asarray(inputs["b_atom"], np.float32)
    W_edge = np.asarray(inputs["W_edge"], np.float32)
    b_edge = np.asarray(inputs["b_edge"], np.float32)
    W_out = np.asarray(inputs["W_out"], np.float32)
    b_out = np.asarray(inputs["b_out"], np.float32)

    src, dst = ei[0], ei[1]
    order = np.argsort(dst, kind="stable")
    src_s, dst_s = src[order], dst[order]
    ea_s = ea[order]

    blk_of = dst_s // P                       # 0..159
    nblk = NCORES * NB
    counts = np.bincount(blk_of, minlength=nblk)
    starts = np.zeros(nblk + 1, np.int64)
    np.cumsum(counts, out=starts[1:])
    tpb = int(np.ceil(max(1, counts.max()) / P))
    NT = NB * tpb
    EB = NT * P

    # weight folds (same algebra as v1): ke/ve = [h|ea|1] @ w2k/wv2
    Wea = np.concatenate([W_edge, b_edge[None, :]], 0)        # [51, 64]
    w2k = np.zeros((L, XW, H * D), np.float32)
    wv2 = np.zeros((L, XW, H * D), np.float32)
    wqs = np.zeros((L, D + 1, H * D + D), np.float32)
    for l in range(L):
        ew = Wea @ We[l]                                      # [51, 256]
        w2k[l, :D] = Wk[l]
        w2k[l, D:] = ew
        w2k[l, -1] += bk[l]
        wv2[l, :D] = Wv[l] / H
        wv2[l, D:] = ew / H
        wv2[l, -1] += bv[l] / H
        wqs[l, :D, :H * D] = Wq[l]
        wqs[l, D, :H * D] = bq[l]
        wqs[l, :D, H * D:] = Wskip[l]
        wqs[l, D, H * D:] = bskip[l]
    watom = np.concatenate([W_atom, b_atom[None, :]], 0)
    w2k_bf = w2k.astype(ml_dtypes.bfloat16)
    wv2_bf = wv2.astype(ml_dtypes.bfloat16)

    in_maps, g0s = [], []
    for c in range(NCORES):
        n0 = c * NLOC
        xs = np.zeros((NLOC, DA), np.float32)
        real = min(NLOC, max(0, N - n0))
        xs[:real] = x[n0:n0 + real]
        eaT = np.zeros((EB, DE + 1), np.float32)
        srcidx = np.zeros((EB,), np.int32)
        met = np.zeros((EB,), np.float32)
        mask = np.zeros((EB,), np.float32)
        for b in range(NB):
            gb = c * NB + b
            s, e = starts[gb], starts[gb + 1]
            k = e - s
            o = b * tpb * P
            eaT[o:o + k, :DE] = ea_s[s:e]
            eaT[o:o + k, DE] = 1.0
            srcidx[o:o + k] = src_s[s:e]
            met[o:o + k] = dst_s[s:e] - (n0 + b * P)
            mask[o:o + k] = 1.0
        brel = np.full((NLOC, 1), -1.0, np.float32)
        g0 = int(batch[min(n0, N - 1)]) if n0 < N else 0
        if real > 0:
            brel[:real, 0] = batch[n0:n0 + real] - g0
        g0s.append(g0)
        # tile-major [P, NT] layouts: [p, t] = edge t*P + p
        in_maps.append({
            "x_shard": xs,
            "eaT_pad": np.ascontiguousarray(eaT.T).astype(ml_dtypes.bfloat16),
            "srcidx": np.ascontiguousarray(srcidx.reshape(NT, P).T),
            "metmask": np.ascontiguousarray(
                np.stack([met.reshape(NT, P).T,
                          mask.reshape(NT, P).T], axis=2).reshape(P, 2 * NT)),
            "batch_rel": brel,
            "w_atom_aug": watom, "w2k": w2k_bf, "wv2": wv2_bf, "wqs": wqs,
        })

    nc = _build_nc(tpb)
    res = run_bass_kernel_spmd(nc, in_maps, core_ids=list(range(NCORES)))

    sums = np.zeros((G + P, D), np.float64)
    cnts = np.zeros(G + P, np.float64)
    for c in range(NCORES):
        op = res.results[c]["out_pool"]
        sums[g0s[c]:g0s[c] + P] += op[:, :D]
        cnts[g0s[c]:g0s[c] + P] += op[:, D]
    pooled = sums[:G] / np.maximum(cnts[:G], 1.0)[:, None]
    out = pooled.astype(np.float32) @ W_out + b_out
    return out.squeeze()


# revision 3
# speedup vs baseline: 42.8516x; 42.8516x over previous
"""CrystalTransformer (TransformerConv x3 + segment-mean pool) on 8 trn2 cores.

v2: hardware-loop restructure of the one-hot-scatter kernel. Per core:
20 dst blocks x tpb 128-edge tiles. For_i loops over blocks for embed/B1/B2;
only the per-tile indirect h[src] gathers are python-unrolled (indirect DMA
doesn't compile inside For_i on this walrus). q is broadcast edge-wise on-chip
via qt = (S^T)^T @ q_blk (S = one-hot dst_rel matrix, transposed on PE), so no
q gather and no q DRAM round-trip. V is projected per edge (ve = X1 @ Wv2) and
the scatter payload is [ve*ex per head | ex] (260 cols), which removes the
per-head transpose+matmul combine of the old B3. Edge features ship only as
eaT (feature-major, bf16); weights bf16; h stays f32.
"""
import json
import numpy as np
import ml_dtypes

P = 128
N, E, G = 20000, 320000, 256
DA, DE, D, H, L = 92, 50, 64, 4, 3
NCORES = 8
NLOC = 2560            # node slots per core (20 blocks of 128)
NB = NLOC // P         # 20 dst blocks per core
NPAD = NLOC * NCORES   # 20480
XW = D + DE + 1        # 115 = [h_src(64) | ea(50) | 1]
ZW = H * D + H         # 260 = [ve*ex per head | ex]


# ---------------------------------------------------------------- BIR patch --
def _install_birpatch():
    """This container's walrus rejects >1 sem wait per instruction; hoist
    extras onto injected preceding Drains (same engine => same order)."""
    import concourse.bass2jax as b2j
    if getattr(b2j, "_birpatch_installed", False):
        return
    orig = b2j.compile_bir_kernel

    def patch(bir_bytes):
        d = json.loads(bir_bytes)
        for fn in d.get("functions", []):
            for blk in fn.get("blocks", []):
                out = []
                for ins in blk.get("instructions", []):
                    si = ins.get("sync_info") or {}
                    waits = si.get("on_wait") or []
                    if len(waits) > 1:
                        for k, w in enumerate(waits[:-1]):
                            out.append({
                                "debug": ins.get("debug", 0),
                                "engine": ins["engine"], "ins": [], "outs": [],
                                "name": f'{ins["name"]}-w{k}', "opcode": "Drain",
                                "sync_info": {"on_update": [], "on_wait": [w]},
                            })
                        si["on_wait"] = waits[-1:]
                    out.append(ins)
                blk["instructions"] = out
        return json.dumps(d).encode()

    def wrapper(bir_str, *a, **kw):
        try:
            bir_str = patch(bir_str)
        except Exception as e:  # pragma: no cover
            print("[birpatch] failed:", e)
        return orig(bir_str, *a, **kw)

    b2j.compile_bir_kernel = wrapper
    b2j._birpatch_installed = True


# ------------------------------------------------------------------- device --
def _build_nc(tpb):
    import concourse.bass as bass
    import concourse.mybir as mybir
    import concourse.tile as tile
    from concourse.bass import ds
    from concourse.masks import make_identity

    f32, i32, bf16 = mybir.dt.float32, mybir.dt.int32, mybir.dt.bfloat16
    Alu, Act = mybir.AluOpType, mybir.ActivationFunctionType
    NT = NB * tpb          # edge tiles per core
    EB = NT * P            # padded edges per core
    TB = tpb * P           # edges per block

    nc = bass.Bass("TRN2", target_bir_lowering=False, debug=False,
                   num_devices=NCORES)
    di = lambda nm, sh, dt=f32: nc.dram_tensor(nm, sh, dt, kind="ExternalInput")
    x_in = di("x_shard", [NLOC, DA], bf16)
    eaT_in = di("eaT_pad", [DE + 1, EB], bf16)
    srcidx_in = di("srcidx", [P, NT], i32)
    metmask_in = di("metmask", [P, 2 * NT], bf16)  # [dst_rel | mask]
    brel_in = di("batch_rel", [NLOC, 1])
    watom_in = di("w_atom_aug", [DA + 1, D])
    w2k_in = di("w2k", [L, XW, H * D], bf16)
    wv2_in = di("wv2", [L, XW, H * D], bf16)
    wqs_in = di("wqs", [L, D + 1, H * D + D])
    out_pool = nc.dram_tensor("out_pool", [P, D + 1], f32, kind="ExternalOutput")

    h_mine = nc.dram_tensor("h_mine", [NLOC, D], f32)
    h_full = [nc.dram_tensor(f"h_full_{l}", [NPAD, D], f32, addr_space="Shared")
              for l in range(L)]

    with tile.TileContext(nc, num_cores=NCORES) as tc:
        import contextlib
        with contextlib.ExitStack() as st:
            cp = st.enter_context(tc.tile_pool(name="const", bufs=1))
            io = st.enter_context(tc.tile_pool(name="io", bufs=3))
            xp = st.enter_context(tc.tile_pool(name="xt", bufs=3))
            vp = st.enter_context(tc.tile_pool(name="dve", bufs=3))
            ps_tr = st.enter_context(tc.tile_pool(name="ps_tr", bufs=2, space="PSUM"))
            ps_ke = st.enter_context(tc.tile_pool(name="ps_ke", bufs=1, space="PSUM"))
            ps_ve = st.enter_context(tc.tile_pool(name="ps_ve", bufs=1, space="PSUM"))
            ps_qt = st.enter_context(tc.tile_pool(name="ps_qt", bufs=1, space="PSUM"))
            ps_z = st.enter_context(tc.tile_pool(name="ps_z", bufs=2, space="PSUM"))
            ps_b = st.enter_context(tc.tile_pool(name="ps_b", bufs=1, space="PSUM"))

            ident = cp.tile([P, P], f32)
            make_identity(nc, ident[:])
            ident_bf = cp.tile([P, P], bf16)
            nc.vector.tensor_copy(ident_bf[:], ident[:])
            iota_i = cp.tile([P, P], i32)
            nc.gpsimd.iota(iota_i[:], pattern=[[1, P]], base=0, channel_multiplier=0)
            iota_f = cp.tile([P, P], f32)
            nc.vector.tensor_copy(iota_f[:], iota_i[:])
            ones_col = cp.tile([P, 1], f32)
            nc.vector.memset(ones_col[:], 1.0)
            srcidx = cp.tile([P, NT], i32)
            nc.sync.dma_start(out=srcidx[:], in_=srcidx_in[:])
            metmask = cp.tile([P, 2 * NT], bf16)
            nc.sync.dma_start(out=metmask[:], in_=metmask_in[:])
            watom_sb = cp.tile([DA + 1, D], f32)
            nc.sync.dma_start(out=watom_sb[:], in_=watom_in[:])
            X_all = cp.tile([P, NT * D], f32)         # 87KB/part

            # ---- embed: h0 = x@W_atom + b
            with tc.For_i(0, NB, 1) as b:
                xb = io.tile([P, DA], bf16, tag="xb")
                nc.sync.dma_start(out=xb[:], in_=x_in[ds(b * P, P), :])
                xT_ps = ps_tr.tile([DA, P], bf16, tag="tr")
                nc.tensor.transpose(out=xT_ps[:], in_=xb[:], identity=ident_bf[:])
                xT = xp.tile([DA + 1, P], f32, tag="xat")
                nc.vector.memset(xT[:], 1.0)
                nc.vector.tensor_copy(xT[:DA, :], xT_ps[:])
                hb_ps = ps_b.tile([P, H * D + D], f32, tag="blk")
                nc.tensor.matmul(hb_ps[:, :D], lhsT=xT[:], rhs=watom_sb[:],
                                 start=True, stop=True)
                hb = vp.tile([P, D], f32, tag="hb")
                nc.vector.tensor_copy(hb[:], hb_ps[:, :D])
                nc.sync.dma_start(out=h_mine[ds(b * P, P), :], in_=hb[:])
            tc.strict_bb_all_engine_barrier()
            nc.gpsimd.collective_compute(
                "AllGather", Alu.bypass,
                replica_groups=[list(range(NCORES))],
                ins=[h_mine.ap().opt()], outs=[h_full[0].ap().opt()])
            tc.strict_bb_all_engine_barrier()

            for l in range(L):
                w2k_sb = cp.tile([XW, H * D], bf16, tag="w2k")
                nc.sync.dma_start(out=w2k_sb[:], in_=w2k_in[l])
                wv2_sb = cp.tile([XW, H * D], bf16, tag="wv2")
                nc.sync.dma_start(out=wv2_sb[:], in_=wv2_in[l])
                wqs_sb = cp.tile([D + 1, H * D + D], f32, tag="wqs")
                nc.sync.dma_start(out=wqs_sb[:], in_=wqs_in[l])

                # ---- G: gather h[src] per edge tile (HW honors 128 offsets
                # per indirect DMA; multi-column offset APs silently gather
                # only the first column's rows)
                for t in range(NT):
                    nc.gpsimd.indirect_dma_start(
                        out=X_all[:, t * D:(t + 1) * D], out_offset=None,
                        in_=h_full[l][:],
                        in_offset=bass.IndirectOffsetOnAxis(
                            ap=srcidx[:, t:t + 1], axis=0))

                # ---- B2: q/skip + edge tiles + combine, per block
                with tc.For_i(0, NB, 1) as b:
                    hblk = io.tile([P, D], f32, tag="hblk")
                    nc.sync.dma_start(out=hblk[:], in_=h_mine[ds(b * P, P), :])
                    hT_ps = ps_tr.tile([D, P], f32, tag="tr")
                    nc.tensor.transpose(out=hT_ps[:], in_=hblk[:], identity=ident[:])
                    hT = xp.tile([D + 1, P], f32, tag="hat")
                    nc.vector.memset(hT[:], 1.0)
                    nc.vector.tensor_copy(hT[:D, :], hT_ps[:])
                    qs_ps = ps_b.tile([P, H * D + D], f32, tag="blk")
                    nc.tensor.matmul(qs_ps[:], lhsT=hT[:], rhs=wqs_sb[:],
                                     start=True, stop=True)
                    qb = xp.tile([P, H * D], f32, tag="qb")
                    nc.scalar.copy(qb[:], qs_ps[:, :H * D])
                    skip_b = xp.tile([P, D], f32, tag="skip_b")
                    nc.scalar.copy(skip_b[:], qs_ps[:, H * D:])
                    mm_blk = vp.tile([P, 2 * tpb], f32, tag="mm_blk")
                    nc.vector.tensor_copy(
                        mm_blk[:], metmask[:, ds(b * (2 * tpb), 2 * tpb)])
                    X_blk = vp.tile([P, tpb * D], f32, tag="X_blk")
                    nc.vector.tensor_copy(
                        X_blk[:], X_all[:, ds(b * (tpb * D), tpb * D)])
                    eaT_blk = io.tile([DE + 1, TB], bf16, tag="eaT_blk")
                    nc.sync.dma_start(out=eaT_blk[:],
                                      in_=eaT_in[:, ds(b * TB, TB)])
                    z_ps = ps_z.tile([P, ZW], f32, tag="z")
                    for t in range(tpb):
                        XT = xp.tile([XW, P], bf16, tag="XT")
                        nc.scalar.copy(XT[D:, :], eaT_blk[:, t * P:(t + 1) * P])
                        hsT_ps = ps_tr.tile([D, P], f32, tag="tr")
                        nc.tensor.transpose(out=hsT_ps[:],
                                            in_=X_blk[:, t * D:(t + 1) * D],
                                            identity=ident[:])
                        nc.vector.tensor_copy(XT[:D, :], hsT_ps[:])
                        ke_ps = ps_ke.tile([P, H * D], f32, tag="ke")
                        nc.tensor.matmul(ke_ps[:], lhsT=XT[:], rhs=w2k_sb[:],
                                         start=True, stop=True)
                        ve_ps = ps_ve.tile([P, H * D], f32, tag="ve")
                        nc.tensor.matmul(ve_ps[:], lhsT=XT[:], rhs=wv2_sb[:],
                                         start=True, stop=True)
                        S = vp.tile([P, P], f32, tag="S")
                        nc.gpsimd.tensor_scalar(
                            out=S[:], in0=iota_f[:],
                            scalar1=mm_blk[:, 2 * t:2 * t + 1],
                            scalar2=mm_blk[:, 2 * t + 1:2 * t + 2],
                            op0=Alu.is_equal, op1=Alu.mult)
                        ST_ps = ps_tr.tile([P, P], f32, tag="tr")
                        nc.tensor.transpose(out=ST_ps[:], in_=S[:],
                                            identity=ident[:])
                        ST = vp.tile([P, P], f32, tag="ST")
                        nc.scalar.copy(ST[:], ST_ps[:])
                        qt_ps = ps_qt.tile([P, H * D], f32, tag="qt")
                        nc.tensor.matmul(qt_ps[:], lhsT=ST[:], rhs=qb[:],
                                         start=True, stop=True)
                        qt = vp.tile([P, H * D], f32, tag="qtc")
                        nc.scalar.copy(qt[:], qt_ps[:])
                        prod = vp.tile([P, H * D], f32, tag="prod")
                        nc.vector.tensor_tensor(out=prod[:], in0=ke_ps[:],
                                                in1=qt[:], op=Alu.mult)
                        alpha = vp.tile([P, H], f32, tag="alpha")
                        nc.vector.tensor_reduce(
                            out=alpha[:],
                            in_=prod[:].rearrange("p (h d) -> p h d", d=D),
                            axis=mybir.AxisListType.X, op=Alu.add)
                        ex = vp.tile([P, H], f32, tag="ex")
                        nc.scalar.activation(ex[:], alpha[:], Act.Exp,
                                             scale=float(1.0 / np.sqrt(D)))
                        veex = vp.tile([P, ZW], f32, tag="veex")
                        for h in range(H):
                            nc.vector.tensor_scalar_mul(
                                out=veex[:, h * D:(h + 1) * D],
                                in0=ve_ps[:, h * D:(h + 1) * D],
                                scalar1=ex[:, h:h + 1])
                        nc.vector.tensor_copy(veex[:, H * D:], ex[:])
                        nc.tensor.matmul(z_ps[:], lhsT=S[:], rhs=veex[:],
                                         start=(t == 0), stop=(t == tpb - 1))
                    # ---- B3: combine per block
                    den = vp.tile([P, H], f32, tag="den")
                    nc.vector.tensor_scalar_max(out=den[:], in0=z_ps[:, H * D:],
                                                scalar1=1e-30)
                    rden = vp.tile([P, H], f32, tag="rden")
                    nc.vector.reciprocal(rden[:], den[:])
                    zn = vp.tile([P, H * D], f32, tag="zn")
                    for h in range(H):
                        nc.vector.tensor_scalar_mul(
                            out=zn[:, h * D:(h + 1) * D],
                            in0=z_ps[:, h * D:(h + 1) * D],
                            scalar1=rden[:, h:h + 1])
                    agg = vp.tile([P, D], f32, tag="agg")
                    nc.vector.tensor_reduce(
                        out=agg[:],
                        in_=zn[:].rearrange("p (h d) -> p d h", h=H),
                        axis=mybir.AxisListType.X, op=Alu.add)
                    tmp = vp.tile([P, D], f32, tag="tmp")
                    nc.vector.tensor_tensor(out=tmp[:], in0=agg[:],
                                            in1=skip_b[:], op=Alu.add)
                    hb2 = vp.tile([P, D], f32, tag="hb2")
                    nc.vector.tensor_scalar_max(out=hb2[:], in0=tmp[:],
                                                scalar1=0.0)
                    nc.sync.dma_start(out=h_mine[ds(b * P, P), :], in_=hb2[:])
                if l < L - 1:
                    tc.strict_bb_all_engine_barrier()
                    nc.gpsimd.collective_compute(
                        "AllGather", Alu.bypass,
                        replica_groups=[list(range(NCORES))],
                        ins=[h_mine.ap().opt()], outs=[h_full[l + 1].ap().opt()])
                    tc.strict_bb_all_engine_barrier()

            tc.strict_bb_all_engine_barrier()
            # ---- pooling: one-hot on batch ids (python-unrolled, PSUM accum)
            brel = cp.tile([P, NB], f32)
            nc.sync.dma_start(out=brel[:],
                              in_=brel_in[:].rearrange("(b p) o -> p (b o)", p=P))
            pool_ps = ps_z.tile([P, D], f32, tag="z")
            cnt_ps = ps_qt.tile([P, 1], f32, tag="qt")
            for b in range(NB):
                hpb = io.tile([P, D], f32, tag="hpb")
                nc.sync.dma_start(out=hpb[:], in_=h_mine[b * P:(b + 1) * P, :])
                Sb = vp.tile([P, P], f32, tag="S")
                nc.vector.tensor_scalar(out=Sb[:], in0=iota_f[:],
                                        scalar1=brel[:, b:b + 1], scalar2=None,
                                        op0=Alu.is_equal)
                nc.tensor.matmul(pool_ps[:], lhsT=Sb[:], rhs=hpb[:],
                                 start=(b == 0), stop=(b == NB - 1))
                nc.tensor.matmul(cnt_ps[:], lhsT=Sb[:], rhs=ones_col[:],
                                 start=(b == 0), stop=(b == NB - 1),
                                 skip_group_check=True)
            pool_sb = vp.tile([P, D + 1], f32, tag="pool_sb")
            nc.vector.tensor_copy(pool_sb[:, :D], pool_ps[:])
            nc.vector.tensor_copy(pool_sb[:, D:], cnt_ps[:])
            nc.sync.dma_start(out=out_pool[:], in_=pool_sb[:])
    return nc


# --------------------------------------------------------------------- host --
def kernel(**inputs):
    _install_birpatch()
    from concourse.bass_utils import run_bass_kernel_spmd

    x = np.asarray(inputs["x"], np.float32)
    ei = np.asarray(inputs["edge_index"]).astype(np.int64)
    ea = np.asarray(inputs["edge_attr"], np.float32)
    batch = np.asarray(inputs["batch"]).astype(np.int64)
    Wq = np.asarray(inputs["Wq"], np.float32); bq = np.asarray(inputs["bq"], np.float32)
    Wk = np.asarray(inputs["Wk"], np.float32); bk = np.asarray(inputs["bk"], np.float32)
    Wv = np.asarray(inputs["Wv"], np.float32); bv = np.asarray(inputs["bv"], np.float32)
    We = np.asarray(inputs["We"], np.float32)
    Wskip = np.asarray(inputs["Wskip"], np.float32)
    bskip = np.asarray(inputs["bskip"], np.float32)
    W_atom = np.asarray(inputs["W_atom"], np.float32)
    b_atom = np.asarray(inputs["b_atom"], np.float32)
    W_edge = np.asarray(inputs["W_edge"], np.float32)
    b_edge = np.asarray(inputs["b_edge"], np.float32)
    W_out = np.asarray(inputs["W_out"], np.float32)
    b_out = np.asarray(inputs["b_out"], np.float32)

    src, dst = ei[0], ei[1]
    order = np.argsort(dst, kind="stable")
    src_s, dst_s = src[order], dst[order]
    ea_s = ea[order]

    blk_of = dst_s // P                       # 0..159
    nblk = NCORES * NB
    counts = np.bincount(blk_of, minlength=nblk)
    starts = np.zeros(nblk + 1, np.int64)
    np.cumsum(counts, out=starts[1:])
    tpb = int(np.ceil(max(1, counts.max()) / P))
    NT = NB * tpb
    EB = NT * P

    # weight folds (same algebra as v1): ke/ve = [h|ea|1] @ w2k/wv2
    Wea = np.concatenate([W_edge, b_edge[None, :]], 0)        # [51, 64]
    w2k = np.zeros((L, XW, H * D), np.float32)
    wv2 = np.zeros((L, XW, H * D), np.float32)
    wqs = np.zeros((L, D + 1, H * D + D), np.float32)
    for l in range(L):
        ew = Wea @ We[l]                                      # [51, 256]
        w2k[l, :D] = Wk[l]
        w2k[l, D:] = ew
        w2k[l, -1] += bk[l]
        wv2[l, :D] = Wv[l] / H
        wv2[l, D:] = ew / H
        wv2[l, -1] += bv[l] / H
        wqs[l, :D, :H * D] = Wq[l]
        wqs[l, D, :H * D] = bq[l]
        wqs[l, :D, H * D:] = Wskip[l]
        wqs[l, D, H * D:] = bskip[l]
    watom = np.concatenate([W_atom, b_atom[None, :]], 0)
    w2k_bf = w2k.astype(ml_dtypes.bfloat16)
    wv2_bf = wv2.astype(ml_dtypes.bfloat16)

    in_maps, g0s = [], []
    for c in range(NCORES):
        n0 = c * NLOC
        xs = np.zeros((NLOC, DA), np.float32)
        real = min(NLOC, max(0, N - n0))
        xs[:real] = x[n0:n0 + real]
        eaT = np.zeros((EB, DE + 1), np.float32)
        srcidx = np.zeros((EB,), np.int32)
        met = np.zeros((EB,), np.float32)
        mask = np.zeros((EB,), np.float32)
        for b in range(NB):
            gb = c * NB + b
            s, e = starts[gb], starts[gb + 1]
            k = e - s
            o = b * tpb * P
            eaT[o:o + k, :DE] = ea_s[s:e]
            eaT[o:o + k, DE] = 1.0
            srcidx[o:o + k] = src_s[s:e]
            met[o:o + k] = dst_s[s:e] - (n0 + b * P)
            mask[o:o + k] = 1.0
        brel = np.full((NLOC, 1), -1.0, np.float32)
        g0 = int(batch[min(n0, N - 1)]) if n0 < N else 0
        if real > 0:
            brel[:real, 0] = batch[n0:n0 + real] - g0
        g0s.append(g0)
        # tile-major [P, NT] layouts: [p, t] = edge t*P + p
        in_maps.append({
            "x_shard": xs.astype(ml_dtypes.bfloat16),
            "eaT_pad": np.ascontiguousarray(eaT.T).astype(ml_dtypes.bfloat16),
            "srcidx": np.ascontiguousarray(srcidx.reshape(NT, P).T),
            "metmask": np.ascontiguousarray(
                np.stack([met.reshape(NT, P).T,
                          mask.reshape(NT, P).T],
                         axis=2).reshape(P, 2 * NT)).astype(ml_dtypes.bfloat16),
            "batch_rel": brel,
            "w_atom_aug": watom, "w2k": w2k_bf, "wv2": wv2_bf, "wqs": wqs,
        })

    nc = _build_nc(tpb)
    res = run_bass_kernel_spmd(nc, in_maps, core_ids=list(range(NCORES)))

    sums = np.zeros((G + P, D), np.float64)
    cnts = np.zeros(G + P, np.float64)
    for c in range(NCORES):
        op = res.results[c]["out_pool"]
        sums[g0s[c]:g0s[c] + P] += op[:, :D]
        cnts[g0s[c]:g0s[c] + P] += op[:, D]
    pooled = sums[:G] / np.maximum(cnts[:G], 1.0)[:, None]
    out = pooled.astype(np.float32) @ W_out + b_out
    return out.squeeze()
